# revision 23
# baseline (speedup 1.0000x reference)
"""Trainium2 Bass kernel for nn_Difference (ignorematch mode).

Math: result[i,j] = sum_k a_fk[i,k] * (a_fk[i,k] > 0) * (b_fk[j,k] <= 0)
where a_fk = a @ feats.T, b_fk = b @ feats.T.  This factorizes into three
matmuls with elementwise ops between them:

    P = relu(a @ feats.T)            # [Na, K]
    Q = (b @ feats.T) <= 0           # [Nb, K], exactly {0.0, 1.0}
    result = P @ Q.T                 # [Na, Nb]

No [Na, Nb, K] tensor is ever materialized.

Sharding: 4x2 grid over the output. Core (r, q) computes
result[r*256:(r+1)*256, q*512:(q+1)*512] from a-quarter r and b-half q;
feats is replicated.  JB=512 makes the b-side and final matmuls full
512-column (one PSUM bank) passes, so LDWEIGHTS is always hidden.

Precision: everything runs in fp16.  The mask side (b_fk sign) flips on
|b_fk| < ~0.04: measured on the fixed test inputs this flips 18 of 262k
mask bits; together with fp16 a-side/output rounding the result's norm
rel err is 2.2e-3 vs the 2e-2 gate.  PE fp16 is 1 col/cycle vs fp32's
2 cycles/col/pass * 2 passes, and halves every DMA byte.

DMA plan.  Each DMA completion pays a ~2.4us HBM-receipt that serializes
per ring; only the two HWDGE rings (Sync/Scalar) are usable (SWDGE
measured 7us+ for 256KB under load).  The ACT engine is NOT used at all:
any activation hoists a ~1.3us act-table load onto the Scalar queue head,
delaying that ring's first DMA.  All PSUM eviction runs on DVE.  Inputs
split evenly, one packed DMA per ring (512KB each - smaller transfers
also tighten the per-core HBM-arbitration variance that sets the
max-over-cores time):
  Sync   HWDGE: d-chunks 0,1 of packed feats.T+b.T AND of a.T
  Scalar HWDGE: d-chunks 2,3 of the same
Outputs leave as two 128KB fp16 pieces, one per ring (receipts overlap);
host upcasts to f32 during the unshard.

PE warmup: the HAM clock gate keeps the PE at ~1.2 GHz until it has been
busy-without-gaps for ~5.5us; any PE idle gap delays the 2.4 GHz ramp
(measured across runs: gap-free flips at start+5.5us, two gaps pushed it
to start+10us).  So: the warm tile is memset by the otherwise-idle
GpSimd engine (its preamble retires ~0.8us before Vector's), dummy
matmuls bridge until the input semaphores, and the b-side (chunks in
ring arrival order) runs before the a-side, with every elementwise dep
produced one loop-group ahead of the matmul that consumes it.
"""

import os
import sys

import numpy as np

sys.path.insert(0, "/opt/trn_rl_repo")

import concourse.bacc as bacc  # noqa: E402
import concourse.tile as tile  # noqa: E402
from concourse import mybir  # noqa: E402
from concourse.bass_utils import run_bass_kernel_spmd  # noqa: E402

# Problem shapes (hardcoded per contract).
NA, NB, D, K = 1024, 1024, 512, 256
A_SPLIT, B_SPLIT = 4, 2  # 8 cores in a 4x2 grid over the output
IA = NA // A_SPLIT  # 256 output rows per core
JB = NB // B_SPLIT  # 512 output cols per core
P = 128
DC = D // P  # 4 contraction chunks
KC = K // P  # 2 feature-bank chunks
MC = IA // P  # 2 output row chunks
FB = K + JB  # packed feats+b row length per (partition, dc): 768
HW = 2 * FB + 2 * IA  # one ring's packed row: fb rows then ah rows (2048)

F32 = mybir.dt.float32
F16 = mybir.dt.float16

# Dummy 256-col matmuls bridging from engine-preamble end until the input
# DMAs land.  Sized for the WORST-case DMA latency (~4.7us after issue):
# warmup start and DMA issue share the same preamble clock, so a span that
# covers the p99 data+receipt time leaves every core gap-free.  A PE gap
# costs ~1.5us (idle + delayed HAM clock ramp); overshoot costs only the
# overshoot, and only on cores that aren't the max anyway.
N_WARM = 22

_BUILT = None
LAST_RESULTS = None


def _build():
    nc = bacc.Bacc("TRN2", target_bir_lowering=False, debug=False)

    # Per-ring packed inputs, one contiguous 4KB run per partition:
    # inH[p, 0:FB]       = [feats.T ; b.T half] rows for d-chunk 2H
    # inH[p, FB:2FB]     = same for d-chunk 2H+1
    # inH[p, 2FB:2FB+IA] = a.T quarter rows for d-chunk 2H
    # inH[p, 2FB+IA:]    = same for d-chunk 2H+1
    in0 = nc.dram_tensor("in0", [P, HW], F16, kind="ExternalInput")
    in1 = nc.dram_tensor("in1", [P, HW], F16, kind="ExternalInput")
    out = nc.dram_tensor("out", [P, MC, JB], F16, kind="ExternalOutput")

    with tile.TileContext(nc) as tc:
        with (
            tc.tile_pool(name="ins", bufs=1) as in_pool,
            tc.tile_pool(name="mid", bufs=1) as mid_pool,
            tc.tile_pool(name="outs", bufs=1) as out_pool,
            tc.tile_pool(name="ps_w", bufs=1, space="PSUM") as ps_w_pool,
            tc.tile_pool(name="ps_b", bufs=2, space="PSUM") as ps_b_pool,
            tc.tile_pool(name="ps_a", bufs=2, space="PSUM") as ps_a_pool,
            tc.tile_pool(name="ps_o", bufs=2, space="PSUM") as ps_o_pool,
        ):
            in_sb = [
                in_pool.tile([P, HW], F16, tag=f"in{h}", name=f"in_sb{h}")
                for h in range(2)
            ]
            # One DMA per HWDGE ring, first instruction on each engine.
            nc.sync.dma_start(out=in_sb[0][:], in_=in0[:])
            nc.scalar.dma_start(out=in_sb[1][:], in_=in1[:])

            # PE clock warmup while the DMAs fly.  A half-width warm tile
            # keeps the memset short (the framework requires the tile be
            # written before the PE reads it), so the first warmup matmul
            # issues as early as Vector clears its preamble.
            warm_sb = in_pool.tile([P, 256], F16, tag="warm", name="warm_sb")
            nc.vector.memset(warm_sb[:], 0.0)
            ps_w = ps_w_pool.tile([P, 256], F32, tag="psw", name="ps_w")
            for _ in range(N_WARM):
                nc.tensor.matmul(
                    ps_w[:], lhsT=warm_sb[:, 0:P], rhs=warm_sb[:], start=True, stop=True
                )

            def fT(dc, kc):  # feats.T chunk [128d, 128k]
                return in_sb[dc // 2][:, (dc % 2) * FB + kc * P :][:, 0:P]

            def bT(dc):  # b.T chunk [128d, 512j]
                return in_sb[dc // 2][:, (dc % 2) * FB + K :][:, 0:JB]

            def aT(dc):  # a.T chunk [128d, 256i]
                return in_sb[dc // 2][:, 2 * FB + (dc % 2) * IA :][:, 0:IA]

            QT_sb = mid_pool.tile([P, KC, JB], F16, tag="qt")
            PT_sb = mid_pool.tile([P, KC, IA], F16, tag="pt")
            out_sb = out_pool.tile([P, MC, JB], F16, tag="osb")

            ps_b = [
                ps_b_pool.tile([P, JB], F32, tag="psb", name=f"ps_b{kc}")
                for kc in range(KC)
            ]
            ps_a = [
                ps_a_pool.tile([P, IA], F32, tag="psa", name=f"ps_a{kc}")
                for kc in range(KC)
            ]
            ps_o = [
                ps_o_pool.tile([P, JB], F32, tag="pso", name=f"ps_o{mc}")
                for mc in range(MC)
            ]

            # Ring-phased accumulation: ALL of ring0's d-chunks (b-side and
            # a-side partial sums, ~2.5us of matmuls) run before anything
            # from ring1, so a late second DMA cannot stall the PE
            # mid-stream (per-core HBM arbitration makes one ring ~1-2us
            # late on some core most runs, and a PE gap also delays the
            # HAM clock ramp).  PSUM accumulation groups per bank pause
            # across the interleave (start on d0, stop on d3).
            for dc in (0, 1):  # ring0 phase
                for kc in range(KC):
                    nc.tensor.matmul(
                        ps_b[kc][:],
                        lhsT=fT(dc, kc),
                        rhs=bT(dc),
                        start=(dc == 0),
                        stop=False,
                    )
                for kc in range(KC):
                    nc.tensor.matmul(
                        ps_a[kc][:],
                        lhsT=fT(dc, kc),
                        rhs=aT(dc),
                        start=(dc == 0),
                        stop=False,
                    )
            # ring1 phase, ordered so the elementwise dep chain of the
            # finals starts as early as possible: b-k0 closes first
            # (isle0 on DVE), both a-side groups close next (relu0, relu1
            # on ACT -- its ~0.35us dispatch lag serializes, so it needs
            # the head start), b-k1 closes last (isle1 on DVE runs while
            # the k0 finals stream).  relu on ACT: the act-table load this
            # hoists onto the Scalar queue delays the in1 DMA ~1.3us --
            # absorbed by the ring-phased schedule.
            def close_b(kc):
                for dc in (2, 3):
                    nc.tensor.matmul(
                        ps_b[kc][:],
                        lhsT=fT(dc, kc),
                        rhs=bT(dc),
                        start=False,
                        stop=(dc == 3),
                    )
                nc.vector.tensor_scalar(
                    QT_sb[:, kc, :], ps_b[kc][:], 0.0, None, mybir.AluOpType.is_le
                )

            def close_a(kc):
                for dc in (2, 3):
                    nc.tensor.matmul(
                        ps_a[kc][:],
                        lhsT=fT(dc, kc),
                        rhs=aT(dc),
                        start=False,
                        stop=(dc == 3),
                    )
                nc.scalar.activation(
                    PT_sb[:, kc, :], ps_a[kc][:], mybir.ActivationFunctionType.Relu
                )

            close_b(0)
            close_a(0)
            close_a(1)
            close_b(1)

            # Finals: out[i,j] = sum_k PT[k,i] * QT[k,j], 512-col passes.
            for kc in range(KC):
                for mc in range(MC):
                    nc.tensor.matmul(
                        ps_o[mc][:],
                        lhsT=PT_sb[:, kc, mc * P : (mc + 1) * P],
                        rhs=QT_sb[:, kc, :],
                        start=(kc == 0),
                        stop=(kc == KC - 1),
                    )

            # Evict (cast f32->fp16, DVE and ACT whole-piece in parallel --
            # finer splits lose to ACT's ~0.5us dispatch lag) + store, one
            # piece per ring.
            nc.vector.tensor_copy(out_sb[:, 0, :], ps_o[0][:])
            nc.sync.dma_start(out=out[:, 0, :], in_=out_sb[:, 0, :])
            nc.scalar.activation(
                out_sb[:, 1, :], ps_o[1][:], mybir.ActivationFunctionType.Copy
            )
            nc.scalar.dma_start(out=out[:, 1, :], in_=out_sb[:, 1, :])

    nc.finalize()
    return nc


def kernel(a, b, feats):
    global _BUILT, LAST_RESULTS
    a = np.ascontiguousarray(a, dtype=np.float32)
    b = np.ascontiguousarray(b, dtype=np.float32)
    feats = np.ascontiguousarray(feats, dtype=np.float32)

    if _BUILT is None:
        _BUILT = _build()
    nc = _BUILT

    fT_r = np.ascontiguousarray(feats.T).astype(np.float16).reshape(DC, P, K)
    bT_r = np.ascontiguousarray(b.T).astype(np.float16).reshape(DC, P, NB)
    aT_r = np.ascontiguousarray(a.T).astype(np.float16).reshape(DC, P, NA)

    in_maps = []
    for r in range(A_SPLIT):
        for q in range(B_SPLIT):
            ins = {}
            for h in range(2):
                buf = np.empty((P, HW), dtype=np.float16)
                for j in range(2):
                    dc = 2 * h + j
                    buf[:, j * FB : j * FB + K] = fT_r[dc]
                    buf[:, j * FB + K : (j + 1) * FB] = bT_r[
                        dc, :, q * JB : (q + 1) * JB
                    ]
                    buf[:, 2 * FB + j * IA : 2 * FB + (j + 1) * IA] = aT_r[
                        dc, :, r * IA : (r + 1) * IA
                    ]
                ins[f"in{h}"] = buf
            in_maps.append(ins)

    kwargs = {}
    if os.environ.get("KERNEL_TRACE"):
        try:
            import antenv.axon_hooks  # noqa: F401  (shimmed by test.py)

            kwargs = dict(trace=True, trace_cores=list(range(8)))
        except ImportError:
            pass
    res = run_bass_kernel_spmd(nc, in_maps, core_ids=list(range(8)), **kwargs)
    LAST_RESULTS = res

    out = np.empty((NA, NB), dtype=np.float32)
    for c, r_map in enumerate(res.results):
        r, q = divmod(c, B_SPLIT)
        # device out: [P, MC, JB]; rows of result tile are mc*128 + p
        tile_out = r_map["out"].transpose(1, 0, 2).reshape(IA, JB).astype(np.float32)
        out[r * IA : (r + 1) * IA, q * JB : (q + 1) * JB] = tile_out
    return out


# revision 24
# speedup vs baseline: 1.0073x; 1.0073x over previous
"""Trainium2 Bass kernel for nn_Difference (ignorematch mode).

Math: result[i,j] = sum_k a_fk[i,k] * (a_fk[i,k] > 0) * (b_fk[j,k] <= 0)
where a_fk = a @ feats.T, b_fk = b @ feats.T.  This factorizes into three
matmuls with elementwise ops between them:

    P = relu(a @ feats.T)            # [Na, K]
    Q = (b @ feats.T) <= 0           # [Nb, K], exactly {0.0, 1.0}
    result = P @ Q.T                 # [Na, Nb]

No [Na, Nb, K] tensor is ever materialized.

Sharding: 4x2 grid over the output. Core (r, q) computes
result[r*256:(r+1)*256, q*512:(q+1)*512] from a-quarter r and b-half q;
feats is replicated.  JB=512 makes the b-side and final matmuls full
512-column (one PSUM bank) passes, so LDWEIGHTS is always hidden.

Precision: everything runs in fp16.  The mask side (b_fk sign) flips on
|b_fk| < ~0.04: measured on the fixed test inputs this flips 18 of 262k
mask bits; together with fp16 a-side/output rounding the result's norm
rel err is 2.2e-3 vs the 2e-2 gate.  PE fp16 is 1 col/cycle vs fp32's
2 cycles/col/pass * 2 passes, and halves every DMA byte.

DMA plan.  Each DMA completion pays a ~2.4us HBM-receipt that serializes
per ring; only the two HWDGE rings (Sync/Scalar) are usable (SWDGE
measured 7us+ for 256KB under load).  The ACT engine is NOT used at all:
any activation hoists a ~1.3us act-table load onto the Scalar queue head,
delaying that ring's first DMA.  All PSUM eviction runs on DVE.  Inputs
split evenly, one packed DMA per ring (512KB each - smaller transfers
also tighten the per-core HBM-arbitration variance that sets the
max-over-cores time):
  Sync   HWDGE: d-chunks 0,1 of packed feats.T+b.T AND of a.T
  Scalar HWDGE: d-chunks 2,3 of the same
Outputs leave as two 128KB fp16 pieces, one per ring (receipts overlap);
host upcasts to f32 during the unshard.

PE warmup: the HAM clock gate keeps the PE at ~1.2 GHz until it has been
busy-without-gaps for ~5.5us; any PE idle gap delays the 2.4 GHz ramp
(measured across runs: gap-free flips at start+5.5us, two gaps pushed it
to start+10us).  So: the warm tile is memset by the otherwise-idle
GpSimd engine (its preamble retires ~0.8us before Vector's), dummy
matmuls bridge until the input semaphores, and the b-side (chunks in
ring arrival order) runs before the a-side, with every elementwise dep
produced one loop-group ahead of the matmul that consumes it.
"""

import os
import sys

import numpy as np

sys.path.insert(0, "/opt/trn_rl_repo")

import concourse.bacc as bacc  # noqa: E402
import concourse.tile as tile  # noqa: E402
from concourse import mybir  # noqa: E402
from concourse.bass_utils import run_bass_kernel_spmd  # noqa: E402

# Problem shapes (hardcoded per contract).
NA, NB, D, K = 1024, 1024, 512, 256
A_SPLIT, B_SPLIT = 4, 2  # 8 cores in a 4x2 grid over the output
IA = NA // A_SPLIT  # 256 output rows per core
JB = NB // B_SPLIT  # 512 output cols per core
P = 128
DC = D // P  # 4 contraction chunks
KC = K // P  # 2 feature-bank chunks
MC = IA // P  # 2 output row chunks
FB = K + JB  # packed feats+b row length per (partition, dc): 768
HW = 2 * FB + 2 * IA  # one ring's packed row: fb rows then ah rows (2048)

F32 = mybir.dt.float32
F16 = mybir.dt.float16

# Dummy 256-col matmuls bridging from engine-preamble end until the input
# DMAs land.  Sized for the WORST-case DMA latency (~4.7us after issue):
# warmup start and DMA issue share the same preamble clock, so a span that
# covers the p99 data+receipt time leaves every core gap-free.  A PE gap
# costs ~1.5us (idle + delayed HAM clock ramp); overshoot costs only the
# overshoot, and only on cores that aren't the max anyway.
N_WARM = 22

_BUILT = None
LAST_RESULTS = None


def _build():
    nc = bacc.Bacc("TRN2", target_bir_lowering=False, debug=False)

    # Per-ring packed inputs, one contiguous 4KB run per partition:
    # inH[p, 0:FB]       = [feats.T ; b.T half] rows for d-chunk 2H
    # inH[p, FB:2FB]     = same for d-chunk 2H+1
    # inH[p, 2FB:2FB+IA] = a.T quarter rows for d-chunk 2H
    # inH[p, 2FB+IA:]    = same for d-chunk 2H+1
    in0 = nc.dram_tensor("in0", [P, HW], F16, kind="ExternalInput")
    in1 = nc.dram_tensor("in1", [P, HW], F16, kind="ExternalInput")
    out = nc.dram_tensor("out", [P, MC, JB], F16, kind="ExternalOutput")

    with tile.TileContext(nc) as tc:
        with (
            tc.tile_pool(name="ins", bufs=1) as in_pool,
            tc.tile_pool(name="mid", bufs=1) as mid_pool,
            tc.tile_pool(name="outs", bufs=1) as out_pool,
            tc.tile_pool(name="ps_w", bufs=1, space="PSUM") as ps_w_pool,
            tc.tile_pool(name="ps_b", bufs=2, space="PSUM") as ps_b_pool,
            tc.tile_pool(name="ps_a", bufs=2, space="PSUM") as ps_a_pool,
            tc.tile_pool(name="ps_o", bufs=2, space="PSUM") as ps_o_pool,
        ):
            in_sb = [
                in_pool.tile([P, HW], F16, tag=f"in{h}", name=f"in_sb{h}")
                for h in range(2)
            ]
            # One DMA per HWDGE ring, first instruction on each engine.
            nc.sync.dma_start(out=in_sb[0][:], in_=in0[:])
            nc.scalar.dma_start(out=in_sb[1][:], in_=in1[:])

            # PE clock warmup while the DMAs fly.  A half-width warm tile
            # keeps the memset short (the framework requires the tile be
            # written before the PE reads it), so the first warmup matmul
            # issues as early as Vector clears its preamble.
            warm_sb = in_pool.tile([P, 256], F16, tag="warm", name="warm_sb")
            nc.vector.memset(warm_sb[:], 0.0)
            ps_w = ps_w_pool.tile([P, 256], F32, tag="psw", name="ps_w")
            for _ in range(N_WARM):
                nc.tensor.matmul(
                    ps_w[:], lhsT=warm_sb[:, 0:P], rhs=warm_sb[:], start=True, stop=True
                )

            def fT(dc, kc):  # feats.T chunk [128d, 128k]
                return in_sb[dc // 2][:, (dc % 2) * FB + kc * P :][:, 0:P]

            def bT(dc):  # b.T chunk [128d, 512j]
                return in_sb[dc // 2][:, (dc % 2) * FB + K :][:, 0:JB]

            def aT(dc):  # a.T chunk [128d, 256i]
                return in_sb[dc // 2][:, 2 * FB + (dc % 2) * IA :][:, 0:IA]

            QT_sb = mid_pool.tile([P, KC, JB], F16, tag="qt")
            PT_sb = mid_pool.tile([P, KC, IA], F16, tag="pt")
            out_sb = out_pool.tile([P, MC, JB], F16, tag="osb")

            ps_b = [
                ps_b_pool.tile([P, JB], F32, tag="psb", name=f"ps_b{kc}")
                for kc in range(KC)
            ]
            ps_a = [
                ps_a_pool.tile([P, IA], F32, tag="psa", name=f"ps_a{kc}")
                for kc in range(KC)
            ]
            ps_o = [
                ps_o_pool.tile([P, JB], F32, tag="pso", name=f"ps_o{mc}")
                for mc in range(MC)
            ]

            # Ring-phased accumulation: ALL of ring0's d-chunks (b-side and
            # a-side partial sums, ~2.5us of matmuls) run before anything
            # from ring1, so a late second DMA cannot stall the PE
            # mid-stream (per-core HBM arbitration makes one ring ~1-2us
            # late on some core most runs, and a PE gap also delays the
            # HAM clock ramp).  PSUM accumulation groups per bank pause
            # across the interleave (start on d0, stop on d3).
            for dc in (0, 1):  # ring0 phase
                for kc in range(KC):
                    nc.tensor.matmul(
                        ps_b[kc][:],
                        lhsT=fT(dc, kc),
                        rhs=bT(dc),
                        start=(dc == 0),
                        stop=False,
                    )
                for kc in range(KC):
                    nc.tensor.matmul(
                        ps_a[kc][:],
                        lhsT=fT(dc, kc),
                        rhs=aT(dc),
                        start=(dc == 0),
                        stop=False,
                    )
            # ring1 phase, ordered so the elementwise dep chain of the
            # finals starts as early as possible: b-k0 closes first
            # (isle0 on DVE), both a-side groups close next (relu0, relu1
            # on ACT -- its ~0.35us dispatch lag serializes, so it needs
            # the head start), b-k1 closes last (isle1 on DVE runs while
            # the k0 finals stream).  relu on ACT: the act-table load this
            # hoists onto the Scalar queue delays the in1 DMA ~1.3us --
            # absorbed by the ring-phased schedule.
            def close_b(kc):
                for dc in (2, 3):
                    nc.tensor.matmul(
                        ps_b[kc][:],
                        lhsT=fT(dc, kc),
                        rhs=bT(dc),
                        start=False,
                        stop=(dc == 3),
                    )
                nc.vector.tensor_scalar(
                    QT_sb[:, kc, :], ps_b[kc][:], 0.0, None, mybir.AluOpType.is_le
                )

            def close_a(kc):
                for dc in (2, 3):
                    nc.tensor.matmul(
                        ps_a[kc][:],
                        lhsT=fT(dc, kc),
                        rhs=aT(dc),
                        start=False,
                        stop=(dc == 3),
                    )
                nc.scalar.activation(
                    PT_sb[:, kc, :], ps_a[kc][:], mybir.ActivationFunctionType.Relu
                )

            close_b(0)
            close_b(1)
            close_a(0)
            close_a(1)

            # Finals: out[i,j] = sum_k PT[k,i] * QT[k,j], 512-col passes.
            for kc in range(KC):
                for mc in range(MC):
                    nc.tensor.matmul(
                        ps_o[mc][:],
                        lhsT=PT_sb[:, kc, mc * P : (mc + 1) * P],
                        rhs=QT_sb[:, kc, :],
                        start=(kc == 0),
                        stop=(kc == KC - 1),
                    )

            # Evict (cast f32->fp16, DVE and ACT whole-piece in parallel --
            # finer splits lose to ACT's ~0.5us dispatch lag) + store, one
            # piece per ring.
            nc.vector.tensor_copy(out_sb[:, 0, :], ps_o[0][:])
            nc.sync.dma_start(out=out[:, 0, :], in_=out_sb[:, 0, :])
            nc.scalar.activation(
                out_sb[:, 1, :], ps_o[1][:], mybir.ActivationFunctionType.Copy
            )
            nc.scalar.dma_start(out=out[:, 1, :], in_=out_sb[:, 1, :])

    nc.finalize()
    return nc


def kernel(a, b, feats):
    global _BUILT, LAST_RESULTS
    a = np.ascontiguousarray(a, dtype=np.float32)
    b = np.ascontiguousarray(b, dtype=np.float32)
    feats = np.ascontiguousarray(feats, dtype=np.float32)

    if _BUILT is None:
        _BUILT = _build()
    nc = _BUILT

    fT_r = np.ascontiguousarray(feats.T).astype(np.float16).reshape(DC, P, K)
    bT_r = np.ascontiguousarray(b.T).astype(np.float16).reshape(DC, P, NB)
    aT_r = np.ascontiguousarray(a.T).astype(np.float16).reshape(DC, P, NA)

    in_maps = []
    for r in range(A_SPLIT):
        for q in range(B_SPLIT):
            ins = {}
            for h in range(2):
                buf = np.empty((P, HW), dtype=np.float16)
                for j in range(2):
                    dc = 2 * h + j
                    buf[:, j * FB : j * FB + K] = fT_r[dc]
                    buf[:, j * FB + K : (j + 1) * FB] = bT_r[
                        dc, :, q * JB : (q + 1) * JB
                    ]
                    buf[:, 2 * FB + j * IA : 2 * FB + (j + 1) * IA] = aT_r[
                        dc, :, r * IA : (r + 1) * IA
                    ]
                ins[f"in{h}"] = buf
            in_maps.append(ins)

    kwargs = {}
    if os.environ.get("KERNEL_TRACE"):
        try:
            import antenv.axon_hooks  # noqa: F401  (shimmed by test.py)

            kwargs = dict(trace=True, trace_cores=list(range(8)))
        except ImportError:
            pass
    res = run_bass_kernel_spmd(nc, in_maps, core_ids=list(range(8)), **kwargs)
    LAST_RESULTS = res

    out = np.empty((NA, NB), dtype=np.float32)
    for c, r_map in enumerate(res.results):
        r, q = divmod(c, B_SPLIT)
        # device out: [P, MC, JB]; rows of result tile are mc*128 + p
        tile_out = r_map["out"].transpose(1, 0, 2).reshape(IA, JB).astype(np.float32)
        out[r * IA : (r + 1) * IA, q * JB : (q + 1) * JB] = tile_out
    return out


# revision 26
# speedup vs baseline: 1.0224x; 1.0150x over previous
"""Trainium2 Bass kernel for nn_Difference (ignorematch mode).

Math: result[i,j] = sum_k a_fk[i,k] * (a_fk[i,k] > 0) * (b_fk[j,k] <= 0)
where a_fk = a @ feats.T, b_fk = b @ feats.T.  This factorizes into three
matmuls with elementwise ops between them:

    P = relu(a @ feats.T)            # [Na, K]
    Q = (b @ feats.T) <= 0           # [Nb, K], exactly {0.0, 1.0}
    result = P @ Q.T                 # [Na, Nb]

No [Na, Nb, K] tensor is ever materialized.

Sharding: 4x2 grid over the output. Core (r, q) computes
result[r*256:(r+1)*256, q*512:(q+1)*512] from a-quarter r and b-half q;
feats is replicated.  JB=512 makes the b-side and final matmuls full
512-column (one PSUM bank) passes, so LDWEIGHTS is always hidden.

Precision: everything runs in fp16.  The mask side (b_fk sign) flips on
|b_fk| < ~0.04: measured on the fixed test inputs this flips 18 of 262k
mask bits; together with fp16 a-side/output rounding the result's norm
rel err is 2.2e-3 vs the 2e-2 gate.  PE fp16 is 1 col/cycle vs fp32's
2 cycles/col/pass * 2 passes, and halves every DMA byte.

DMA plan.  Each DMA completion pays a ~2.4us HBM-receipt that serializes
per ring; only the two HWDGE rings (Sync/Scalar) are usable (SWDGE
measured 7us+ for 256KB under load).  The ACT engine is NOT used at all:
any activation hoists a ~1.3us act-table load onto the Scalar queue head,
delaying that ring's first DMA.  All PSUM eviction runs on DVE.  Inputs
split evenly, one packed DMA per ring (512KB each - smaller transfers
also tighten the per-core HBM-arbitration variance that sets the
max-over-cores time):
  Sync   HWDGE: d-chunks 0,1 of packed feats.T+b.T AND of a.T
  Scalar HWDGE: d-chunks 2,3 of the same
Outputs leave as two 128KB fp16 pieces, one per ring (receipts overlap);
host upcasts to f32 during the unshard.

PE warmup: the HAM clock gate keeps the PE at ~1.2 GHz until it has been
busy-without-gaps for ~5.5us; any PE idle gap delays the 2.4 GHz ramp
(measured across runs: gap-free flips at start+5.5us, two gaps pushed it
to start+10us).  Dummy matmuls bridge from preamble end until the input
semaphores land on every core (see N_WARM), and the ring-phased main
loop keeps the PE gap-free from there.

Measured on the fixed test inputs: ~21.6-22.3us max-over-cores (chip
thermal state adds ~±0.5us; sustained back-to-back runs throttle the
whole chip ~15-20%), vs the 26.9us fp32-b-side baseline.
"""

import os
import sys

import numpy as np

sys.path.insert(0, "/opt/trn_rl_repo")

import concourse.bacc as bacc  # noqa: E402
import concourse.tile as tile  # noqa: E402
from concourse import mybir  # noqa: E402
from concourse.bass_utils import run_bass_kernel_spmd  # noqa: E402

# Problem shapes (hardcoded per contract).
NA, NB, D, K = 1024, 1024, 512, 256
A_SPLIT, B_SPLIT = 4, 2  # 8 cores in a 4x2 grid over the output
IA = NA // A_SPLIT  # 256 output rows per core
JB = NB // B_SPLIT  # 512 output cols per core
P = 128
DC = D // P  # 4 contraction chunks
KC = K // P  # 2 feature-bank chunks
MC = IA // P  # 2 output row chunks
FB = K + JB  # packed feats+b row length per (partition, dc): 768
HW = 2 * FB + 2 * IA  # one ring's packed row: fb rows then ah rows (2048)

F32 = mybir.dt.float32
F16 = mybir.dt.float16

# Dummy 256-col matmuls bridging from engine-preamble end until the input
# DMAs land.  Sized for the WORST-case DMA latency (~4.7us after issue):
# warmup start and DMA issue share the same preamble clock, so a span that
# covers the p99 data+receipt time leaves every core gap-free.  A PE gap
# costs ~1.5us (idle + delayed HAM clock ramp); overshoot costs only the
# overshoot, and only on cores that aren't the max anyway.
N_WARM = 22

_BUILT = None
LAST_RESULTS = None


def _build():
    nc = bacc.Bacc("TRN2", target_bir_lowering=False, debug=False)

    # Per-ring packed inputs, one contiguous 4KB run per partition:
    # inH[p, 0:FB]       = [feats.T ; b.T half] rows for d-chunk 2H
    # inH[p, FB:2FB]     = same for d-chunk 2H+1
    # inH[p, 2FB:2FB+IA] = a.T quarter rows for d-chunk 2H
    # inH[p, 2FB+IA:]    = same for d-chunk 2H+1
    in0 = nc.dram_tensor("in0", [P, HW], F16, kind="ExternalInput")
    in1 = nc.dram_tensor("in1", [P, HW], F16, kind="ExternalInput")
    out = nc.dram_tensor("out", [P, MC, JB], F16, kind="ExternalOutput")

    with tile.TileContext(nc) as tc:
        with (
            tc.tile_pool(name="ins", bufs=1) as in_pool,
            tc.tile_pool(name="mid", bufs=1) as mid_pool,
            tc.tile_pool(name="outs", bufs=1) as out_pool,
            tc.tile_pool(name="ps_w", bufs=1, space="PSUM") as ps_w_pool,
            tc.tile_pool(name="ps_b", bufs=2, space="PSUM") as ps_b_pool,
            tc.tile_pool(name="ps_a", bufs=2, space="PSUM") as ps_a_pool,
            tc.tile_pool(name="ps_o", bufs=2, space="PSUM") as ps_o_pool,
        ):
            in_sb = [
                in_pool.tile([P, HW], F16, tag=f"in{h}", name=f"in_sb{h}")
                for h in range(2)
            ]
            # One DMA per HWDGE ring, first instruction on each engine.
            nc.sync.dma_start(out=in_sb[0][:], in_=in0[:])
            nc.scalar.dma_start(out=in_sb[1][:], in_=in1[:])

            # PE clock warmup while the DMAs fly.  A half-width warm tile
            # keeps the memset short (the framework requires the tile be
            # written before the PE reads it), so the first warmup matmul
            # issues as early as Vector clears its preamble.
            warm_sb = in_pool.tile([P, 256], F16, tag="warm", name="warm_sb")
            nc.vector.memset(warm_sb[:], 0.0)
            ps_w = ps_w_pool.tile([P, 256], F32, tag="psw", name="ps_w")
            for _ in range(N_WARM):
                nc.tensor.matmul(
                    ps_w[:], lhsT=warm_sb[:, 0:P], rhs=warm_sb[:], start=True, stop=True
                )

            def fT(dc, kc):  # feats.T chunk [128d, 128k]
                return in_sb[dc // 2][:, (dc % 2) * FB + kc * P :][:, 0:P]

            def bT(dc):  # b.T chunk [128d, 512j]
                return in_sb[dc // 2][:, (dc % 2) * FB + K :][:, 0:JB]

            def aT(dc):  # a.T chunk [128d, 256i]
                return in_sb[dc // 2][:, 2 * FB + (dc % 2) * IA :][:, 0:IA]

            QT_sb = mid_pool.tile([P, KC, JB], F16, tag="qt")
            PT_sb = mid_pool.tile([P, KC, IA], F16, tag="pt")
            out_sb = out_pool.tile([P, MC, JB], F16, tag="osb")

            ps_b = [
                ps_b_pool.tile([P, JB], F32, tag="psb", name=f"ps_b{kc}")
                for kc in range(KC)
            ]
            ps_a = [
                ps_a_pool.tile([P, IA], F32, tag="psa", name=f"ps_a{kc}")
                for kc in range(KC)
            ]
            ps_o = [
                ps_o_pool.tile([P, JB], F32, tag="pso", name=f"ps_o{mc}")
                for mc in range(MC)
            ]

            # Ring-phased accumulation: ALL of ring0's d-chunks (b-side and
            # a-side partial sums, ~2.5us of matmuls) run before anything
            # from ring1, so a late second DMA cannot stall the PE
            # mid-stream (per-core HBM arbitration makes one ring ~1-2us
            # late on some core most runs, and a PE gap also delays the
            # HAM clock ramp).  PSUM accumulation groups per bank pause
            # across the interleave (start on d0, stop on d3).
            for dc in (0, 1):  # ring0 phase
                for kc in range(KC):
                    nc.tensor.matmul(
                        ps_b[kc][:],
                        lhsT=fT(dc, kc),
                        rhs=bT(dc),
                        start=(dc == 0),
                        stop=False,
                    )
                for kc in range(KC):
                    nc.tensor.matmul(
                        ps_a[kc][:],
                        lhsT=fT(dc, kc),
                        rhs=aT(dc),
                        start=(dc == 0),
                        stop=False,
                    )
            # ring1 phase: both b-side groups close first (QT feeds every
            # final; is_le on DVE hides under the following matmuls), then
            # the a-side groups close with relu on ACT right behind.  Any
            # close order ends within ~0.1us of this one -- the tail is
            # floored by the engines' ~0.3us dispatch lag plus two serial
            # 0.7us is_le ops on DVE.  relu on ACT: the act-table load it
            # hoists onto the Scalar queue delays the in1 DMA ~1.3us --
            # absorbed by the ring-phased schedule.
            def close_b(kc):
                for dc in (2, 3):
                    nc.tensor.matmul(
                        ps_b[kc][:],
                        lhsT=fT(dc, kc),
                        rhs=bT(dc),
                        start=False,
                        stop=(dc == 3),
                    )
                nc.vector.tensor_scalar(
                    QT_sb[:, kc, :], ps_b[kc][:], 0.0, None, mybir.AluOpType.is_le
                )

            def close_a(kc):
                for dc in (2, 3):
                    nc.tensor.matmul(
                        ps_a[kc][:],
                        lhsT=fT(dc, kc),
                        rhs=aT(dc),
                        start=False,
                        stop=(dc == 3),
                    )
                nc.scalar.activation(
                    PT_sb[:, kc, :], ps_a[kc][:], mybir.ActivationFunctionType.Relu
                )

            close_b(0)
            close_b(1)
            close_a(0)
            close_a(1)

            # Finals: out[i,j] = sum_k PT[k,i] * QT[k,j], 512-col passes.
            for kc in range(KC):
                for mc in range(MC):
                    nc.tensor.matmul(
                        ps_o[mc][:],
                        lhsT=PT_sb[:, kc, mc * P : (mc + 1) * P],
                        rhs=QT_sb[:, kc, :],
                        start=(kc == 0),
                        stop=(kc == KC - 1),
                    )

            # Evict (cast f32->fp16, DVE and ACT whole-piece in parallel --
            # finer splits lose to ACT's ~0.5us dispatch lag) + store, one
            # piece per ring.
            nc.vector.tensor_copy(out_sb[:, 0, :], ps_o[0][:])
            nc.sync.dma_start(out=out[:, 0, :], in_=out_sb[:, 0, :])
            nc.scalar.activation(
                out_sb[:, 1, :], ps_o[1][:], mybir.ActivationFunctionType.Copy
            )
            nc.scalar.dma_start(out=out[:, 1, :], in_=out_sb[:, 1, :])

    nc.finalize()
    return nc


def kernel(a, b, feats):
    global _BUILT, LAST_RESULTS
    a = np.ascontiguousarray(a, dtype=np.float32)
    b = np.ascontiguousarray(b, dtype=np.float32)
    feats = np.ascontiguousarray(feats, dtype=np.float32)

    if _BUILT is None:
        _BUILT = _build()
    nc = _BUILT

    fT_r = np.ascontiguousarray(feats.T).astype(np.float16).reshape(DC, P, K)
    bT_r = np.ascontiguousarray(b.T).astype(np.float16).reshape(DC, P, NB)
    aT_r = np.ascontiguousarray(a.T).astype(np.float16).reshape(DC, P, NA)

    in_maps = []
    for r in range(A_SPLIT):
        for q in range(B_SPLIT):
            ins = {}
            for h in range(2):
                buf = np.empty((P, HW), dtype=np.float16)
                for j in range(2):
                    dc = 2 * h + j
                    buf[:, j * FB : j * FB + K] = fT_r[dc]
                    buf[:, j * FB + K : (j + 1) * FB] = bT_r[
                        dc, :, q * JB : (q + 1) * JB
                    ]
                    buf[:, 2 * FB + j * IA : 2 * FB + (j + 1) * IA] = aT_r[
                        dc, :, r * IA : (r + 1) * IA
                    ]
                ins[f"in{h}"] = buf
            in_maps.append(ins)

    kwargs = {}
    if os.environ.get("KERNEL_TRACE"):
        try:
            import antenv.axon_hooks  # noqa: F401  (shimmed by test.py)

            kwargs = dict(trace=True, trace_cores=list(range(8)))
        except ImportError:
            pass
    res = run_bass_kernel_spmd(nc, in_maps, core_ids=list(range(8)), **kwargs)
    LAST_RESULTS = res

    out = np.empty((NA, NB), dtype=np.float32)
    for c, r_map in enumerate(res.results):
        r, q = divmod(c, B_SPLIT)
        # device out: [P, MC, JB]; rows of result tile are mc*128 + p
        tile_out = r_map["out"].transpose(1, 0, 2).reshape(IA, JB).astype(np.float32)
        out[r * IA : (r + 1) * IA, q * JB : (q + 1) * JB] = tile_out
    return out
